# revision 12
# baseline (speedup 1.0000x reference)
"""Trainium2 Bass kernel for DFlashAttention (draft/target cross-attention).

Sharding: TP=2 over heads (16 q heads / 4 kv heads per core) x DP=4 over batch.
Core c = tp*4 + b. Each core computes a partial output [64, 2048] (its 16 heads
through its slice of Wo); the host sums the two TP partials per batch.

Per-core pipeline:
  - hidden/target activations are cast to bf16 and transposed via the DMA xbar
    (d on partitions) to feed the projections.
  - K/V projection in bf16 (fp32 accumulation in PSUM), RMS-norm + RoPE on K in
    fp32, K^T materialized in bf16 for the score matmuls, V kept natural (bf16).
  - Scores per q-head pair (GQA group shares kv head), softmax without
    max-subtraction (scores are provably bounded ~|s|<=32 for rms-normed q/k
    with cos/sin in [0,1]), probs normalized then transposed on the PE for the
    PV matmul; output projection in fp32r.
"""

import os
import numpy as np
from contextlib import ExitStack

B, QL, CTX, KV, D = 4, 64, 4096, 4160, 2048
H, KVH, HD = 32, 8, 64
TP, DP = 2, 4
HL, KVHL = H // TP, KVH // TP          # 16 q heads, 4 kv heads per core
GL = HL // 4                           # 4 GQA groups per core (4 q heads each)
DCH = D // 128                         # 16 contraction chunks
NT = CTX // 128                        # 32 ctx kv tiles
EPS = 1e-6

_NC = None
LAST_RESULT = None


def _patch_drain_split():
    """The walrus build in this container rejects >1 sync-wait on a TPB_CTRL
    Drain; split the TileContext final-drain waits across single-wait NOPs."""
    import concourse.tile as tile
    import concourse.mybir as mybir
    from concourse.vector_clock import ScopedClock

    if getattr(tile.TileContext, "_drain_split_patched", False):
        return

    def _drain_and_barrier(self, tick_clock, wait_clock):
        nc = self.nc
        drain_inst = nc.sync.drain()
        wait_clock.add_sem_waits(
            drain_inst.ins, ScopedClock({None: tick_clock.global_clock})
        )
        si = drain_inst.ins.sync_info
        if si is not None and len(si.on_wait) > 1:
            waits = list(si.on_wait)
            si.on_wait = []
            for w in waits:
                n = nc.sync.nop(nofuse=True, hint="drain_wait_split")
                n.ins.sync_info = mybir.SyncInfo(on_wait=[w], on_update=[])
        nc.all_engine_barrier()
        assert self.sems is not None
        popped = nc._tile_sem_poison_stack.pop()
        assert popped is self._sem_poison
        nc.clear_and_free_semaphores(list(self.sems.allocated().values()))
        nc.all_engine_barrier()

    tile.TileContext._drain_and_barrier = _drain_and_barrier
    tile.TileContext._drain_split_patched = True


def _split_excess_waits(nc, mybir, limit=1):
    """The walrus build here allows very few sync-waits per instruction;
    hoist excess waits onto single-wait NOPs on the same engine queue."""
    k = 0
    for f in nc.m.functions:
        for b in f.blocks:
            out = []
            for inst in b.instructions:
                si = inst.sync_info
                if si is not None and len(si.on_wait) > limit:
                    waits = list(si.on_wait)
                    si.on_wait = waits[-limit:]
                    for w in waits[:-limit]:
                        n = mybir.InstEventSemaphore(name=f"I-ws{k}", ins=[], outs=[])
                        k += 1
                        n.engine = inst.engine
                        n.sync_info = mybir.SyncInfo(on_wait=[w], on_update=[])
                        nc.register_instruction(n)
                        out.append(n)
                out.append(inst)
            b.instructions[:] = out


def build_nc():
    import concourse.bass as bass
    import concourse.mybir as mybir
    import concourse.tile as tile
    from concourse.masks import make_identity

    _patch_drain_split()

    f32 = mybir.dt.float32
    f32r = mybir.dt.float32r
    bf16 = mybir.dt.bfloat16
    MUL = mybir.AluOpType.mult
    ADD = mybir.AluOpType.add
    AXX = mybir.AxisListType.X
    EXP = mybir.ActivationFunctionType.Exp

    nc = bass.Bass("TRN2", target_bir_lowering=False, debug=False, num_devices=8)

    hid = nc.dram_tensor("hid", [QL, D], f32, kind="ExternalInput").ap()
    tgt = nc.dram_tensor("tgt", [CTX, D], f32, kind="ExternalInput").ap()
    cosf = nc.dram_tensor("cosf", [KV, HD], f32, kind="ExternalInput").ap()
    sinf = nc.dram_tensor("sinf", [KV, HD], f32, kind="ExternalInput").ap()
    wq = nc.dram_tensor("wq", [D, HL * HD], f32, kind="ExternalInput").ap()
    wk = nc.dram_tensor("wk", [D, KVHL * HD], f32, kind="ExternalInput").ap()
    wv = nc.dram_tensor("wv", [D, KVHL * HD], f32, kind="ExternalInput").ap()
    wo = nc.dram_tensor("wo", [HL * HD, D], f32, kind="ExternalInput").ap()
    qw = nc.dram_tensor("qw", [1, HD], f32, kind="ExternalInput").ap()
    kw = nc.dram_tensor("kw", [1, HD], f32, kind="ExternalInput").ap()
    out = nc.dram_tensor("out", [QL, D], f32, kind="ExternalOutput").ap()

    # score chunks (512-wide) and transpose/PV chunks (128-wide)
    chunks_s = [(c * 512, min(512, KV - c * 512)) for c in range((KV + 511) // 512)]
    chunks_t = [(c * 128, min(128, KV - c * 128)) for c in range((KV + 127) // 128)]

    with tile.TileContext(nc) as tc, ExitStack() as ctx:
        const = ctx.enter_context(tc.tile_pool(name="const", bufs=1))
        kvp = ctx.enter_context(tc.tile_pool(name="kvp", bufs=1))
        xf_pool = ctx.enter_context(tc.tile_pool(name="xf", bufs=2))
        xb_pool = ctx.enter_context(tc.tile_pool(name="xb", bufs=2))
        xt_pool = ctx.enter_context(tc.tile_pool(name="xt", bufs=2))
        kn_pool = ctx.enter_context(tc.tile_pool(name="kn", bufs=2))
        stats = ctx.enter_context(tc.tile_pool(name="stats", bufs=4))
        wq_pool = ctx.enter_context(tc.tile_pool(name="wqp", bufs=2))
        wo_pool = ctx.enter_context(tc.tile_pool(name="wop", bufs=2))
        qp = ctx.enter_context(tc.tile_pool(name="qp", bufs=1))
        ps_big = ctx.enter_context(tc.tile_pool(name="psb", bufs=3, space="PSUM"))

        # ---- constants ----
        IDf = const.tile([128, 128], f32)
        make_identity(nc, IDf)
        IDb = const.tile([128, 128], bf16)
        make_identity(nc, IDb)

        WKV = const.tile([128, DCH, 512], bf16)  # [d128, chunk, k(256)|v(256)]
        for half in range(2):
            sl = slice(half * 8, half * 8 + 8)
            for w_ap, off in ((wk, 0), (wv, 256)):
                scr = xf_pool.tile([128, 2048], f32, tag="xf")
                nc.sync.dma_start(
                    scr[:].rearrange("p (a h) -> p a h", h=256),
                    w_ap.rearrange("(a p) h -> p a h", p=128)[:, sl, :],
                )
                nc.gpsimd.tensor_copy(
                    WKV[:, sl, off : off + 256],
                    scr[:].rearrange("p (a h) -> p a h", h=256),
                )

        COS = const.tile([128, NT, HD], f32)
        nc.sync.dma_start(COS[:], cosf[0:CTX, :].rearrange("(t p) d -> p t d", p=128))
        SINS = const.tile([128, NT, HD], f32)
        scr = xf_pool.tile([128, 2048], f32, tag="xf")
        scr_v = scr[:].rearrange("p (t d) -> p t d", d=HD)
        nc.sync.dma_start(scr_v, sinf[0:CTX, :].rearrange("(t p) d -> p t d", p=128))
        nc.vector.tensor_scalar_mul(SINS[:, :, 0:32], scr_v[:, :, 0:32], -1.0)
        nc.vector.tensor_copy(SINS[:, :, 32:64], scr_v[:, :, 32:64])

        COSD = const.tile([QL, HD], f32)
        nc.sync.dma_start(COSD[:], cosf[CTX:KV, :])
        SINSD = const.tile([QL, HD], f32)
        scrd = stats.tile([QL, HD], f32, tag="scrd")
        nc.sync.dma_start(scrd[:], sinf[CTX:KV, :])
        nc.vector.tensor_scalar_mul(SINSD[:, 0:32], scrd[:, 0:32], -1.0)
        nc.vector.tensor_copy(SINSD[:, 32:64], scrd[:, 32:64])

        # norm weights broadcast across partitions (via K=1 matmul)
        ONES = const.tile([1, 128], f32)
        nc.vector.memset(ONES[:], 1.0)
        EPSB = const.tile([128, 1], f32)
        nc.vector.memset(EPSB[:], EPS)
        KW1 = const.tile([1, HD], f32)
        nc.sync.dma_start(KW1[:], kw)
        QW1 = const.tile([1, HD], f32)
        nc.sync.dma_start(QW1[:], qw)
        NWK = const.tile([128, HD], f32)
        NWQ = const.tile([128, HD], f32)
        for w1, nw in ((KW1, NWK), (QW1, NWQ)):
            psb = ps_big.tile([128, 512], f32, tag="big")
            nc.tensor.matmul(psb[:, 0:HD], lhsT=ONES[:], rhs=w1[:], start=True, stop=True)
            nc.vector.tensor_copy(nw[:], psb[:, 0:HD])

        KT2 = kvp.tile([128, 2, KV], bf16)   # [hd of head pair, pair, kv]
        VN = kvp.tile([128, NT + 1, KVHL * HD], bf16)  # v natural; draft at t=NT rows 0:64

        # ---- helpers ----
        def rms_rope(kq, P, nheads, nw, cos_ap, sins_ap, out_bf, tag):
            """RMS-norm (+weight) + RoPE on a natural-layout tile kq [P, nheads*64]
            (fp32), writing bf16 to out_bf. cos_ap/sins_ap: [P, 64] APs."""
            v3 = kq[:].rearrange("p (h d) -> p h d", d=HD)
            scrn = kn_pool.tile([P, nheads * HD], f32, tag=tag + "scr")
            scr3 = scrn[:].rearrange("p (h d) -> p h d", d=HD)
            mss = stats.tile([P, nheads], f32, tag=tag + "mss")
            nc.vector.tensor_mul(scrn[:], kq[:], kq[:])
            nc.vector.reduce_sum(mss[:, :, None], scr3[:], axis=AXX)
            rms = stats.tile([P, nheads], f32, tag=tag + "rms")
            # rms = sqrt(mean(x^2) + eps) = sqrt(ssq/HD + eps)
            nc.scalar.activation(
                rms[:], mss[:], mybir.ActivationFunctionType.Sqrt,
                bias=EPSB[0:P, :], scale=1.0 / HD,
            )
            riv = stats.tile([P, nheads], f32, tag=tag + "riv")
            nc.vector.reciprocal(riv[:], rms[:])
            for h in range(nheads):
                nc.vector.tensor_scalar(
                    out=v3[:, h, :], in0=v3[:, h, :],
                    scalar1=riv[:, h : h + 1], scalar2=None, op0=MUL,
                )
            nwb = nw[0:P, None, :].broadcast_to([P, nheads, HD])
            nc.vector.tensor_mul(v3[:], v3[:], nwb)
            # rope
            rot = kn_pool.tile([P, nheads * HD], f32, tag=tag + "rot")
            rot3 = rot[:].rearrange("p (h d) -> p h d", d=HD)
            nc.vector.tensor_copy(rot3[:, :, 0:32], v3[:, :, 32:64])
            nc.vector.tensor_copy(rot3[:, :, 32:64], v3[:, :, 0:32])
            cosb = cos_ap[:, None, :].broadcast_to([P, nheads, HD])
            sinsb = sins_ap[:, None, :].broadcast_to([P, nheads, HD])
            nc.vector.tensor_mul(scr3[:], v3[:], cosb)
            nc.vector.tensor_mul(rot3[:], rot3[:], sinsb)
            ob3 = out_bf[:].rearrange("p (h d) -> p h d", d=HD)
            nc.vector.tensor_add(ob3[:], scr3[:], rot3[:])

        # ---- phase 1: q + draft k/v ----
        Hf = xf_pool.tile([QL, D], f32, tag="xf")
        nc.sync.dma_start(Hf[:], hid)
        Hb = xb_pool.tile([QL, D], bf16, tag="xb")
        nc.gpsimd.tensor_copy(Hb[:], Hf[:])
        HTb = qp.tile([128, DCH, QL], bf16)
        nc.sync.dma_start(HTb[:], Hb[:], transpose=True)

        # draft k/v projection
        ps_kvd = ps_big.tile([QL, 512], f32, tag="big")
        for i in range(DCH):
            nc.tensor.matmul(
                ps_kvd[:], lhsT=HTb[:, i, :], rhs=WKV[:, i, :],
                start=(i == 0), stop=(i == DCH - 1),
            )
        Kd = kn_pool.tile([QL, KVHL * HD], f32, tag="kd")
        nc.vector.tensor_copy(Kd[:], ps_kvd[:, 0:256])
        nc.scalar.copy(VN[0:QL, NT, :], ps_kvd[:, 256:512])
        Kdb = kn_pool.tile([QL, KVHL * HD], bf16, tag="kdb")
        rms_rope(Kd, QL, KVHL, NWK, COSD[:], SINSD[:], Kdb, "kd")
        for j in range(2):
            psT = ps_big.tile([128, 512], f32, tag="big")
            psTb = psT[:].bitcast(bf16)
            nc.tensor.transpose(psTb[0:128, 0:QL], Kdb[:, j * 128 : (j + 1) * 128], IDb[0:QL, 0:QL])
            nc.any.tensor_copy(KT2[:, j, CTX:KV], psTb[0:128, 0:QL])

        # q projection (bf16), 2 psum accumulators of 512 cols
        ps_q = [ps_big.tile([QL, 512], f32, tag="big", name=f"ps_q{c}") for c in range(2)]
        for i in range(DCH):
            wqt = wq_pool.tile([128, HL * HD], f32, tag="wqf")
            nc.sync.dma_start(wqt[:], wq[i * 128 : (i + 1) * 128, :])
            wqb = wq_pool.tile([128, HL * HD], bf16, tag="wqb")
            nc.gpsimd.tensor_copy(wqb[:], wqt[:])
            for c in range(2):
                nc.tensor.matmul(
                    ps_q[c][:], lhsT=HTb[:, i, :], rhs=wqb[:, c * 512 : (c + 1) * 512],
                    start=(i == 0), stop=(i == DCH - 1),
                )
        Q = qp.tile([QL, HL * HD], f32)
        for c in range(2):
            nc.vector.tensor_copy(Q[:, c * 512 : (c + 1) * 512], ps_q[c][:])
        Qb = qp.tile([QL, HL * HD], bf16)
        rms_rope(Q, QL, HL, NWQ, COSD[:], SINSD[:], Qb, "q")
        # qT per head -> QTb [64(hd), 16, 64(q)], then duplicate to both
        # partition halves so scores lhsT/rhs share a partition range.
        QTb = qp.tile([QL, HL, QL], bf16)
        for h in range(HL):
            psT = ps_big.tile([128, 512], f32, tag="big")
            psTb = psT[:].bitcast(bf16)
            nc.tensor.transpose(psTb[0:QL, 0:QL], Qb[:, h * 64 : (h + 1) * 64], IDb[0:QL, 0:QL])
            nc.any.tensor_copy(QTb[:, h, :], psTb[0:QL, 0:QL])
        QT2 = qp.tile([128, HL, QL], bf16)
        nc.sync.dma_start(QT2[0:QL, :, :], QTb[:])
        nc.sync.dma_start(QT2[QL:128, :, :], QTb[:])

        # ---- phase 2: ctx K/V over 32 tiles ----
        for t in range(NT):
            Xf = xf_pool.tile([128, D], f32, tag="xf")
            nc.sync.dma_start(Xf[:], tgt[t * 128 : (t + 1) * 128, :])
            Xb = xb_pool.tile([128, D], bf16, tag="xb")
            nc.gpsimd.tensor_copy(Xb[:], Xf[:])
            Xt = xt_pool.tile([128, DCH, 128], bf16, tag="xt")
            nc.sync.dma_start(Xt[:], Xb[:], transpose=True)

            ps_kv = ps_big.tile([128, 512], f32, tag="big")
            for i in range(DCH):
                nc.tensor.matmul(
                    ps_kv[:], lhsT=Xt[:, i, :], rhs=WKV[:, i, :],
                    start=(i == 0), stop=(i == DCH - 1),
                )
            Knat = kn_pool.tile([128, KVHL * HD], f32, tag="knat")
            nc.vector.tensor_copy(Knat[:], ps_kv[:, 0:256])
            nc.scalar.copy(VN[:, t, :], ps_kv[:, 256:512])
            Kb = kn_pool.tile([128, KVHL * HD], bf16, tag="kb")
            rms_rope(Knat, 128, KVHL, NWK, COS[:, t, :], SINS[:, t, :], Kb, "k")
            for j in range(2):
                psT = ps_big.tile([128, 512], f32, tag="big")
                psTb = psT[:].bitcast(bf16)
                nc.tensor.transpose(psTb[0:128, 0:128], Kb[:, j * 128 : (j + 1) * 128], IDb)
                nc.any.tensor_copy(KT2[:, j, t * 128 : (t + 1) * 128], psTb[0:128, 0:128])

        # ---- phase 3: attention per GQA group ----
        OT = qp.tile([QL, HL, HD], f32)  # attn_out^T: [hd, head, q]
        with tc.tile_pool(name="att", bufs=2) as att_pool, \
             tc.tile_pool(name="attt", bufs=3) as attt_pool, \
             tc.tile_pool(name="pst", bufs=3, space="PSUM") as ps_tr, \
             tc.tile_pool(name="pspv", bufs=2, space="PSUM") as ps_pv:
            for g in range(GL):
                pr = 64 * (g % 2)
                jj = g // 2
                att_tiles = []
                for pair in range(2):
                    h0 = 4 * g + 2 * pair
                    ATT = att_pool.tile([128, KV], bf16, tag="att")
                    SUMS = stats.tile([128, 16], f32, tag="sums")
                    for c, (c0, cw) in enumerate(chunks_s):
                        ps_s = ps_big.tile([128, 512], f32, tag="big")
                        nc.tensor.matmul(
                            ps_s[:, 0:cw],
                            lhsT=QT2[pr : pr + 64, h0 : h0 + 2, :],
                            rhs=KT2[pr : pr + 64, jj, c0 : c0 + cw],
                            start=True, stop=True,
                        )
                        nc.scalar.activation(
                            ATT[:, c0 : c0 + cw], ps_s[:, 0:cw], EXP,
                            scale=0.125, accum_out=SUMS[:, c : c + 1],
                        )
                    L = stats.tile([128, 1], f32, tag="lsum")
                    nc.vector.reduce_sum(L[:], SUMS[:, 0 : len(chunks_s)], axis=AXX)
                    LI = stats.tile([128, 1], f32, tag="linv")
                    nc.vector.reciprocal(LI[:], L[:])
                    for c0, cw in chunks_s:
                        nc.vector.tensor_scalar(
                            out=ATT[:, c0 : c0 + cw], in0=ATT[:, c0 : c0 + cw],
                            scalar1=LI[:, 0:1], scalar2=None, op0=MUL,
                        )
                    att_tiles.append(ATT)
                ps_o = ps_pv.tile([QL, 4 * QL], f32, tag="pv")
                for c, (c0, cw) in enumerate(chunks_t):
                    ATTT = attt_pool.tile([128, 256], bf16, tag="attt")
                    for pair in range(2):
                        psT = ps_tr.tile([128, 128], bf16, tag="tr")
                        nc.tensor.transpose(
                            psT[0:cw, 0:128], att_tiles[pair][:, c0 : c0 + cw], IDb
                        )
                        nc.any.tensor_copy(
                            ATTT[0:cw, pair * 128 : (pair + 1) * 128], psT[0:cw, 0:128]
                        )
                    nc.tensor.matmul(
                        ps_o[:],
                        lhsT=VN[0:cw, (c0 // 128), g * 64 : (g + 1) * 64],
                        rhs=ATTT[0:cw, :],
                        start=(c == 0), stop=(c == len(chunks_t) - 1),
                    )
                nc.any.tensor_copy(
                    OT[:, 4 * g : 4 * g + 4, :].rearrange("p h q -> p (h q)"), ps_o[:]
                )

        # ---- phase 4: output projection (fp32r) ----
        f32r_ = f32r
        OUTT = qp.tile([QL, D], f32)
        with tc.tile_pool(name="pso4", bufs=4, space="PSUM") as ps_o4:
            accs = [ps_o4.tile([QL, 512], f32, tag="o4", name=f"acc{c}") for c in range(4)]
            for h in range(HL):
                wot = wo_pool.tile([QL, D], f32, tag="wo")
                nc.sync.dma_start(wot[:], wo[h * 64 : (h + 1) * 64, :])
                for cc in range(4):
                    nc.tensor.matmul(
                        accs[cc][:],
                        lhsT=OT[:, h, :],
                        rhs=wot[:, cc * 512 : (cc + 1) * 512],
                        start=(h == 0), stop=(h == HL - 1),
                    )
            for cc in range(4):
                nc.any.tensor_copy(OUTT[:, cc * 512 : (cc + 1) * 512], accs[cc][:])
        nc.sync.dma_start(out, OUTT[:])

    _split_excess_waits(nc, mybir)
    return nc


def _get_nc():
    global _NC
    if _NC is None:
        _NC = build_nc()
    return _NC


def make_in_maps(hidden_states, target_hidden, cos, sin, Wq, Wk, Wv, Wo,
                 q_norm_w, k_norm_w):
    c = np.ascontiguousarray
    in_maps = []
    for core in range(8):
        tp, b = core // DP, core % DP
        in_maps.append({
            "hid": c(hidden_states[b].astype(np.float32)),
            "tgt": c(target_hidden[b].astype(np.float32)),
            "cosf": c(cos[b].astype(np.float32)),
            "sinf": c(sin[b].astype(np.float32)),
            "wq": c(Wq[:, tp * HL * HD:(tp + 1) * HL * HD].astype(np.float32)),
            "wk": c(Wk[:, tp * KVHL * HD:(tp + 1) * KVHL * HD].astype(np.float32)),
            "wv": c(Wv[:, tp * KVHL * HD:(tp + 1) * KVHL * HD].astype(np.float32)),
            "wo": c(Wo[tp * HL * HD:(tp + 1) * HL * HD, :].astype(np.float32)),
            "qw": c(q_norm_w.reshape(1, HD).astype(np.float32)),
            "kw": c(k_norm_w.reshape(1, HD).astype(np.float32)),
        })
    return in_maps


def kernel(hidden_states, target_hidden, cos, sin, Wq, Wk, Wv, Wo,
           q_norm_w, k_norm_w):
    global LAST_RESULT
    from concourse.bass_utils import run_bass_kernel_spmd

    nc = _get_nc()
    in_maps = make_in_maps(hidden_states, target_hidden, cos, sin,
                           Wq, Wk, Wv, Wo, q_norm_w, k_norm_w)
    trace = os.environ.get("KERNEL_TRACE", "0") == "1"
    res = run_bass_kernel_spmd(nc, in_maps, list(range(8)), trace=trace)
    LAST_RESULT = res
    out = np.zeros((B, QL, D), np.float32)
    for core in range(8):
        tp, b = core // DP, core % DP
        out[b] += res.results[core]["out"]
    return out


# revision 17
# speedup vs baseline: 1.3116x; 1.3116x over previous
"""Trainium2 Bass kernel for DFlashAttention (draft/target cross-attention).

Sharding: TP=2 over heads (16 q heads / 4 kv heads per core) x DP=4 over batch.
Core c = tp*4 + b. Each core computes a partial output [64, 2048] (its 16 heads
through its slice of Wo); the host sums the two TP partials per batch.

Per-core pipeline:
  - hidden/target activations are cast to bf16 and transposed via the DMA xbar
    (d on partitions) to feed the projections.
  - K/V projection in bf16 (fp32 accumulation in PSUM), RMS-norm + RoPE on K in
    fp32, K^T materialized in bf16 for the score matmuls, V kept natural (bf16).
  - Scores per q-head pair (GQA group shares kv head), softmax without
    max-subtraction (scores are provably bounded ~|s|<=32 for rms-normed q/k
    with cos/sin in [0,1]), probs normalized then transposed on the PE for the
    PV matmul; output projection in fp32r.
"""

import os
import numpy as np
from contextlib import ExitStack

B, QL, CTX, KV, D = 4, 64, 4096, 4160, 2048
H, KVH, HD = 32, 8, 64
TP, DP = 2, 4
HL, KVHL = H // TP, KVH // TP          # 16 q heads, 4 kv heads per core
GL = HL // 4                           # 4 GQA groups per core (4 q heads each)
DCH = D // 128                         # 16 contraction chunks
NT = CTX // 128                        # 32 ctx kv tiles
EPS = 1e-6

_NC = None
LAST_RESULT = None


def _patch_drain_split():
    """The walrus build in this container rejects >1 sync-wait on a TPB_CTRL
    Drain; split the TileContext final-drain waits across single-wait NOPs."""
    import concourse.tile as tile
    import concourse.mybir as mybir
    from concourse.vector_clock import ScopedClock

    if getattr(tile.TileContext, "_drain_split_patched", False):
        return

    def _drain_and_barrier(self, tick_clock, wait_clock):
        nc = self.nc
        drain_inst = nc.sync.drain()
        wait_clock.add_sem_waits(
            drain_inst.ins, ScopedClock({None: tick_clock.global_clock})
        )
        si = drain_inst.ins.sync_info
        if si is not None and len(si.on_wait) > 1:
            waits = list(si.on_wait)
            si.on_wait = []
            for w in waits:
                n = nc.sync.nop(nofuse=True, hint="drain_wait_split")
                n.ins.sync_info = mybir.SyncInfo(on_wait=[w], on_update=[])
        nc.all_engine_barrier()
        assert self.sems is not None
        popped = nc._tile_sem_poison_stack.pop()
        assert popped is self._sem_poison
        nc.clear_and_free_semaphores(list(self.sems.allocated().values()))
        nc.all_engine_barrier()

    tile.TileContext._drain_and_barrier = _drain_and_barrier
    tile.TileContext._drain_split_patched = True


def _split_excess_waits(nc, mybir, limit=1):
    """The walrus build here allows very few sync-waits per instruction;
    hoist excess waits onto single-wait NOPs on the same engine queue."""
    k = 0
    for f in nc.m.functions:
        for b in f.blocks:
            out = []
            for inst in b.instructions:
                si = inst.sync_info
                if si is not None and len(si.on_wait) > limit:
                    waits = list(si.on_wait)
                    si.on_wait = waits[-limit:]
                    for w in waits[:-limit]:
                        n = mybir.InstEventSemaphore(name=f"I-ws{k}", ins=[], outs=[])
                        k += 1
                        n.engine = inst.engine
                        n.sync_info = mybir.SyncInfo(on_wait=[w], on_update=[])
                        nc.register_instruction(n)
                        out.append(n)
                out.append(inst)
            b.instructions[:] = out


def build_nc():
    import concourse.bass as bass
    import concourse.mybir as mybir
    import concourse.tile as tile
    from concourse.masks import make_identity

    _patch_drain_split()

    f32 = mybir.dt.float32
    f32r = mybir.dt.float32r
    bf16 = mybir.dt.bfloat16
    MUL = mybir.AluOpType.mult
    ADD = mybir.AluOpType.add
    AXX = mybir.AxisListType.X
    EXP = mybir.ActivationFunctionType.Exp

    nc = bass.Bass("TRN2", target_bir_lowering=False, debug=False, num_devices=8)

    hid = nc.dram_tensor("hid", [QL, D], f32, kind="ExternalInput").ap()
    tgt = nc.dram_tensor("tgt", [CTX, D], f32, kind="ExternalInput").ap()
    cosf = nc.dram_tensor("cosf", [KV, HD], f32, kind="ExternalInput").ap()
    sinf = nc.dram_tensor("sinf", [KV, HD], f32, kind="ExternalInput").ap()
    wq = nc.dram_tensor("wq", [D, HL * HD], f32, kind="ExternalInput").ap()
    wk = nc.dram_tensor("wk", [D, KVHL * HD], f32, kind="ExternalInput").ap()
    wv = nc.dram_tensor("wv", [D, KVHL * HD], f32, kind="ExternalInput").ap()
    wo = nc.dram_tensor("wo", [HL * HD, D], f32, kind="ExternalInput").ap()
    qw = nc.dram_tensor("qw", [1, HD], f32, kind="ExternalInput").ap()
    kw = nc.dram_tensor("kw", [1, HD], f32, kind="ExternalInput").ap()
    out = nc.dram_tensor("out", [QL, D], f32, kind="ExternalOutput").ap()

    # score chunks (512-wide) and transpose/PV chunks (128-wide)
    chunks_s = [(c * 512, min(512, KV - c * 512)) for c in range((KV + 511) // 512)]
    chunks_t = [(c * 128, min(128, KV - c * 128)) for c in range((KV + 127) // 128)]

    with tile.TileContext(nc) as tc, ExitStack() as ctx:
        const = ctx.enter_context(tc.tile_pool(name="const", bufs=1))
        kvp = ctx.enter_context(tc.tile_pool(name="kvp", bufs=1))
        xf_pool = ctx.enter_context(tc.tile_pool(name="xf", bufs=3))
        xt_pool = ctx.enter_context(tc.tile_pool(name="xt", bufs=2))
        kn_pool = ctx.enter_context(tc.tile_pool(name="kn", bufs=2))
        stats = ctx.enter_context(tc.tile_pool(name="stats", bufs=4))
        wq_pool = ctx.enter_context(tc.tile_pool(name="wqp", bufs=2))
        wo_pool = ctx.enter_context(tc.tile_pool(name="wop", bufs=2))
        qp = ctx.enter_context(tc.tile_pool(name="qp", bufs=1))
        ps_big = ctx.enter_context(tc.tile_pool(name="psb", bufs=3, space="PSUM"))
        # transpose psum pool for phases 0-2; closed before the attention phase
        _ps_x_cm = tc.tile_pool(name="psx", bufs=4, space="PSUM")
        ps_x = _ps_x_cm.__enter__()

        # ---- constants ----
        IDf = const.tile([128, 128], f32)
        make_identity(nc, IDf)
        IDb = const.tile([128, 128], bf16)
        make_identity(nc, IDb)

        WKV = const.tile([128, DCH, 512], bf16)  # [d128, chunk, k(256)|v(256)]
        for half in range(2):
            sl = slice(half * 8, half * 8 + 8)
            for w_ap, off in ((wk, 0), (wv, 256)):
                scr = xf_pool.tile([128, 2048], f32, tag="xf")
                nc.sync.dma_start(
                    scr[:].rearrange("p (a h) -> p a h", h=256),
                    w_ap.rearrange("(a p) h -> p a h", p=128)[:, sl, :],
                )
                nc.gpsimd.tensor_copy(
                    WKV[:, sl, off : off + 256],
                    scr[:].rearrange("p (a h) -> p a h", h=256),
                )

        COS = const.tile([128, NT, HD], f32)
        nc.sync.dma_start(COS[:], cosf[0:CTX, :].rearrange("(t p) d -> p t d", p=128))
        SINS = const.tile([128, NT, HD], f32)
        scr = xf_pool.tile([128, 2048], f32, tag="xf")
        scr_v = scr[:].rearrange("p (t d) -> p t d", d=HD)
        nc.sync.dma_start(scr_v, sinf[0:CTX, :].rearrange("(t p) d -> p t d", p=128))
        nc.vector.tensor_scalar_mul(SINS[:, :, 0:32], scr_v[:, :, 0:32], -1.0)
        nc.vector.tensor_copy(SINS[:, :, 32:64], scr_v[:, :, 32:64])

        COSD = const.tile([QL, HD], f32)
        nc.sync.dma_start(COSD[:], cosf[CTX:KV, :])
        SINSD = const.tile([QL, HD], f32)
        scrd = stats.tile([QL, HD], f32, tag="scrd")
        nc.sync.dma_start(scrd[:], sinf[CTX:KV, :])
        nc.vector.tensor_scalar_mul(SINSD[:, 0:32], scrd[:, 0:32], -1.0)
        nc.vector.tensor_copy(SINSD[:, 32:64], scrd[:, 32:64])

        # norm weights broadcast across partitions (via K=1 matmul)
        ONES = const.tile([1, 128], f32)
        nc.vector.memset(ONES[:], 1.0)
        EPSB = const.tile([128, 1], f32)
        nc.vector.memset(EPSB[:], EPS)
        KW1 = const.tile([1, HD], f32)
        nc.sync.dma_start(KW1[:], kw)
        QW1 = const.tile([1, HD], f32)
        nc.sync.dma_start(QW1[:], qw)
        NWK = const.tile([128, HD], f32)
        NWQ = const.tile([128, HD], f32)
        for w1, nw in ((KW1, NWK), (QW1, NWQ)):
            psb = ps_big.tile([128, 512], f32, tag="big")
            nc.tensor.matmul(psb[:, 0:HD], lhsT=ONES[:], rhs=w1[:], start=True, stop=True)
            nc.vector.tensor_copy(nw[:], psb[:, 0:HD])

        KT2 = kvp.tile([128, 2, KV], bf16)   # [hd of head pair, pair, kv]
        VN = kvp.tile([128, NT + 1, KVHL * HD], bf16)  # v natural; draft at t=NT rows 0:64

        # ---- helpers ----
        def rms_rope(kq, P, nheads, nw, cos_ap, sins_ap, out_bf, tag):
            """RMS-norm (+weight) + RoPE on a natural-layout tile kq [P, nheads*64]
            (fp32), writing bf16 to out_bf. cos_ap/sins_ap: [P, 64] APs."""
            v3 = kq[:].rearrange("p (h d) -> p h d", d=HD)
            scrn = kn_pool.tile([P, nheads * HD], f32, tag=tag + "scr")
            scr3 = scrn[:].rearrange("p (h d) -> p h d", d=HD)
            mss = stats.tile([P, nheads], f32, tag=tag + "mss")
            nc.vector.tensor_mul(scrn[:], kq[:], kq[:])
            nc.vector.reduce_sum(mss[:, :, None], scr3[:], axis=AXX)
            rms = stats.tile([P, nheads], f32, tag=tag + "rms")
            # rms = sqrt(mean(x^2) + eps) = sqrt(ssq/HD + eps)
            nc.scalar.activation(
                rms[:], mss[:], mybir.ActivationFunctionType.Sqrt,
                bias=EPSB[0:P, :], scale=1.0 / HD,
            )
            riv = stats.tile([P, nheads], f32, tag=tag + "riv")
            nc.vector.reciprocal(riv[:], rms[:])
            for h in range(nheads):
                nc.vector.tensor_scalar(
                    out=v3[:, h, :], in0=v3[:, h, :],
                    scalar1=riv[:, h : h + 1], scalar2=None, op0=MUL,
                )
            nwb = nw[0:P, None, :].broadcast_to([P, nheads, HD])
            nc.vector.tensor_mul(v3[:], v3[:], nwb)
            # rope
            rot = kn_pool.tile([P, nheads * HD], f32, tag=tag + "rot")
            rot3 = rot[:].rearrange("p (h d) -> p h d", d=HD)
            nc.vector.tensor_copy(rot3[:, :, 0:32], v3[:, :, 32:64])
            nc.vector.tensor_copy(rot3[:, :, 32:64], v3[:, :, 0:32])
            cosb = cos_ap[:, None, :].broadcast_to([P, nheads, HD])
            sinsb = sins_ap[:, None, :].broadcast_to([P, nheads, HD])
            nc.vector.tensor_mul(scr3[:], v3[:], cosb)
            nc.vector.tensor_mul(rot3[:], rot3[:], sinsb)
            ob3 = out_bf[:].rearrange("p (h d) -> p h d", d=HD)
            nc.vector.tensor_add(ob3[:], scr3[:], rot3[:])

        # ---- phase 1: q + draft k/v ----
        Hf = xf_pool.tile([QL, D], f32, tag="xf")
        nc.sync.dma_start(Hf[:], hid)
        HTb = qp.tile([128, DCH, QL], bf16)
        for i in range(DCH):
            psT = ps_x.tile([128, 128], f32, tag="tr")
            nc.tensor.transpose(
                psT[0:128, 0:QL], Hf[:, i * 128 : (i + 1) * 128], IDf[0:QL, 0:QL]
            )
            if i % 2 == 0:
                nc.vector.tensor_copy(HTb[:, i, :], psT[0:128, 0:QL])
            else:
                nc.scalar.copy(HTb[:, i, :], psT[0:128, 0:QL])

        # draft k/v projection
        ps_kvd = ps_big.tile([QL, 512], f32, tag="big")
        for i in range(DCH):
            nc.tensor.matmul(
                ps_kvd[:], lhsT=HTb[:, i, :], rhs=WKV[:, i, :],
                start=(i == 0), stop=(i == DCH - 1),
            )
        Kd = kn_pool.tile([QL, KVHL * HD], f32, tag="kd")
        nc.vector.tensor_copy(Kd[:], ps_kvd[:, 0:256])
        nc.scalar.copy(VN[0:QL, NT, :], ps_kvd[:, 256:512])
        Kdb = kn_pool.tile([QL, KVHL * HD], bf16, tag="kdb")
        rms_rope(Kd, QL, KVHL, NWK, COSD[:], SINSD[:], Kdb, "kd")
        for j in range(2):
            psT = ps_big.tile([128, 512], f32, tag="big")
            psTb = psT[:].bitcast(bf16)
            nc.tensor.transpose(psTb[0:128, 0:QL], Kdb[:, j * 128 : (j + 1) * 128], IDb[0:QL, 0:QL])
            nc.any.tensor_copy(KT2[:, j, CTX:KV], psTb[0:128, 0:QL])

        # q projection (bf16), 2 psum accumulators of 512 cols
        ps_q = [ps_big.tile([QL, 512], f32, tag="big", name=f"ps_q{c}") for c in range(2)]
        for i in range(DCH):
            wqt = wq_pool.tile([128, HL * HD], f32, tag="wqf")
            nc.sync.dma_start(wqt[:], wq[i * 128 : (i + 1) * 128, :])
            wqb = wq_pool.tile([128, HL * HD], bf16, tag="wqb")
            nc.gpsimd.tensor_copy(wqb[:], wqt[:])
            for c in range(2):
                nc.tensor.matmul(
                    ps_q[c][:], lhsT=HTb[:, i, :], rhs=wqb[:, c * 512 : (c + 1) * 512],
                    start=(i == 0), stop=(i == DCH - 1),
                )
        Q = qp.tile([QL, HL * HD], f32)
        for c in range(2):
            nc.vector.tensor_copy(Q[:, c * 512 : (c + 1) * 512], ps_q[c][:])
        Qb = qp.tile([QL, HL * HD], bf16)
        rms_rope(Q, QL, HL, NWQ, COSD[:], SINSD[:], Qb, "q")
        # qT per head -> QTb [64(hd), 16, 64(q)], then duplicate to both
        # partition halves so scores lhsT/rhs share a partition range.
        QTb = qp.tile([QL, HL, QL], bf16)
        for h in range(HL):
            psT = ps_big.tile([128, 512], f32, tag="big")
            psTb = psT[:].bitcast(bf16)
            nc.tensor.transpose(psTb[0:QL, 0:QL], Qb[:, h * 64 : (h + 1) * 64], IDb[0:QL, 0:QL])
            nc.any.tensor_copy(QTb[:, h, :], psTb[0:QL, 0:QL])
        QT2 = qp.tile([128, HL, QL], bf16)
        nc.sync.dma_start(QT2[0:QL, :, :], QTb[:])
        nc.sync.dma_start(QT2[QL:128, :, :], QTb[:])

        # ---- phase 2: ctx K/V over 32 tiles ----
        for t in range(NT):
            Xf = xf_pool.tile([128, D], f32, tag="xf")
            nc.sync.dma_start(Xf[:], tgt[t * 128 : (t + 1) * 128, :])
            Xt = xt_pool.tile([128, DCH, 128], bf16, tag="xt")
            for i in range(DCH):
                psT = ps_x.tile([128, 128], f32, tag="tr")
                nc.tensor.transpose(psT, Xf[:, i * 128 : (i + 1) * 128], IDf)
                if i % 2 == 0:
                    nc.vector.tensor_copy(Xt[:, i, :], psT[:])
                else:
                    nc.scalar.copy(Xt[:, i, :], psT[:])

            ps_kv = ps_big.tile([128, 512], f32, tag="big")
            for i in range(DCH):
                nc.tensor.matmul(
                    ps_kv[:], lhsT=Xt[:, i, :], rhs=WKV[:, i, :],
                    start=(i == 0), stop=(i == DCH - 1),
                )
            Knat = kn_pool.tile([128, KVHL * HD], f32, tag="knat")
            nc.vector.tensor_copy(Knat[:], ps_kv[:, 0:256])
            nc.scalar.copy(VN[:, t, :], ps_kv[:, 256:512])
            Kb = kn_pool.tile([128, KVHL * HD], bf16, tag="kb")
            rms_rope(Knat, 128, KVHL, NWK, COS[:, t, :], SINS[:, t, :], Kb, "k")
            for j in range(2):
                psT = ps_big.tile([128, 512], f32, tag="big")
                psTb = psT[:].bitcast(bf16)
                nc.tensor.transpose(psTb[0:128, 0:128], Kb[:, j * 128 : (j + 1) * 128], IDb)
                nc.any.tensor_copy(KT2[:, j, t * 128 : (t + 1) * 128], psTb[0:128, 0:128])

        # ---- phase 3: attention per GQA group ----
        _ps_x_cm.__exit__(None, None, None)
        OT = qp.tile([QL, HL, HD], f32)  # attn_out^T: [hd, head, q]
        with tc.tile_pool(name="att", bufs=2) as att_pool, \
             tc.tile_pool(name="attt", bufs=3) as attt_pool, \
             tc.tile_pool(name="pst", bufs=3, space="PSUM") as ps_tr, \
             tc.tile_pool(name="pspv", bufs=2, space="PSUM") as ps_pv:
            for g in range(GL):
                pr = 64 * (g % 2)
                jj = g // 2
                att_tiles = []
                for pair in range(2):
                    h0 = 4 * g + 2 * pair
                    ATT = att_pool.tile([128, KV], bf16, tag="att")
                    SUMS = stats.tile([128, 16], f32, tag="sums")
                    for c, (c0, cw) in enumerate(chunks_s):
                        ps_s = ps_big.tile([128, 512], f32, tag="big")
                        nc.tensor.matmul(
                            ps_s[:, 0:cw],
                            lhsT=QT2[pr : pr + 64, h0 : h0 + 2, :],
                            rhs=KT2[pr : pr + 64, jj, c0 : c0 + cw],
                            start=True, stop=True,
                        )
                        nc.scalar.activation(
                            ATT[:, c0 : c0 + cw], ps_s[:, 0:cw], EXP,
                            scale=0.125, accum_out=SUMS[:, c : c + 1],
                        )
                    L = stats.tile([128, 1], f32, tag="lsum")
                    nc.vector.reduce_sum(L[:], SUMS[:, 0 : len(chunks_s)], axis=AXX)
                    LI = stats.tile([128, 1], f32, tag="linv")
                    nc.vector.reciprocal(LI[:], L[:])
                    for c0, cw in chunks_s:
                        nc.vector.tensor_scalar(
                            out=ATT[:, c0 : c0 + cw], in0=ATT[:, c0 : c0 + cw],
                            scalar1=LI[:, 0:1], scalar2=None, op0=MUL,
                        )
                    att_tiles.append(ATT)
                ps_o = ps_pv.tile([QL, 4 * QL], f32, tag="pv")
                for c, (c0, cw) in enumerate(chunks_t):
                    ATTT = attt_pool.tile([128, 256], bf16, tag="attt")
                    for pair in range(2):
                        psT = ps_tr.tile([128, 128], bf16, tag="tr")
                        nc.tensor.transpose(
                            psT[0:cw, 0:128], att_tiles[pair][:, c0 : c0 + cw], IDb
                        )
                        nc.any.tensor_copy(
                            ATTT[0:cw, pair * 128 : (pair + 1) * 128], psT[0:cw, 0:128]
                        )
                    nc.tensor.matmul(
                        ps_o[:],
                        lhsT=VN[0:cw, (c0 // 128), g * 64 : (g + 1) * 64],
                        rhs=ATTT[0:cw, :],
                        start=(c == 0), stop=(c == len(chunks_t) - 1),
                    )
                nc.any.tensor_copy(
                    OT[:, 4 * g : 4 * g + 4, :].rearrange("p h q -> p (h q)"), ps_o[:]
                )

        # ---- phase 4: output projection (fp32r) ----
        f32r_ = f32r
        OUTT = qp.tile([QL, D], f32)
        with tc.tile_pool(name="pso4", bufs=4, space="PSUM") as ps_o4:
            accs = [ps_o4.tile([QL, 512], f32, tag="o4", name=f"acc{c}") for c in range(4)]
            for h in range(HL):
                wot = wo_pool.tile([QL, D], f32, tag="wo")
                nc.sync.dma_start(wot[:], wo[h * 64 : (h + 1) * 64, :])
                for cc in range(4):
                    nc.tensor.matmul(
                        accs[cc][:],
                        lhsT=OT[:, h, :],
                        rhs=wot[:, cc * 512 : (cc + 1) * 512],
                        start=(h == 0), stop=(h == HL - 1),
                    )
            for cc in range(4):
                nc.any.tensor_copy(OUTT[:, cc * 512 : (cc + 1) * 512], accs[cc][:])
        nc.sync.dma_start(out, OUTT[:])

    _split_excess_waits(nc, mybir)
    return nc


def _get_nc():
    global _NC
    if _NC is None:
        _NC = build_nc()
    return _NC


def make_in_maps(hidden_states, target_hidden, cos, sin, Wq, Wk, Wv, Wo,
                 q_norm_w, k_norm_w):
    c = np.ascontiguousarray
    in_maps = []
    for core in range(8):
        tp, b = core // DP, core % DP
        in_maps.append({
            "hid": c(hidden_states[b].astype(np.float32)),
            "tgt": c(target_hidden[b].astype(np.float32)),
            "cosf": c(cos[b].astype(np.float32)),
            "sinf": c(sin[b].astype(np.float32)),
            "wq": c(Wq[:, tp * HL * HD:(tp + 1) * HL * HD].astype(np.float32)),
            "wk": c(Wk[:, tp * KVHL * HD:(tp + 1) * KVHL * HD].astype(np.float32)),
            "wv": c(Wv[:, tp * KVHL * HD:(tp + 1) * KVHL * HD].astype(np.float32)),
            "wo": c(Wo[tp * HL * HD:(tp + 1) * HL * HD, :].astype(np.float32)),
            "qw": c(q_norm_w.reshape(1, HD).astype(np.float32)),
            "kw": c(k_norm_w.reshape(1, HD).astype(np.float32)),
        })
    return in_maps


def kernel(hidden_states, target_hidden, cos, sin, Wq, Wk, Wv, Wo,
           q_norm_w, k_norm_w):
    global LAST_RESULT
    from concourse.bass_utils import run_bass_kernel_spmd

    nc = _get_nc()
    in_maps = make_in_maps(hidden_states, target_hidden, cos, sin,
                           Wq, Wk, Wv, Wo, q_norm_w, k_norm_w)
    trace = os.environ.get("KERNEL_TRACE", "0") == "1"
    res = run_bass_kernel_spmd(nc, in_maps, list(range(8)), trace=trace)
    LAST_RESULT = res
    out = np.zeros((B, QL, D), np.float32)
    for core in range(8):
        tp, b = core // DP, core % DP
        out[b] += res.results[core]["out"]
    return out
